# revision 9
# baseline (speedup 1.0000x reference)
"""CoLA linear kernel for Trainium2: y = x @ kron(U, V) + b.

Math: per token t (16384 of them), with X_t = x[t].reshape(64, 64),
    y[t] = flatten(U^T @ X_t @ V) + b     (row-major flatten, d' = 64*k + l)

v3 design ("fp16 streaming, constants-moving / x-stationary stage 1"):

  - Pure data parallel over tokens: 2048 per NeuronCore x 8 cores.
  - Host casts x to fp16 and pre-permutes it so every DMA descriptor is an
    8 KiB contiguous run (128 descriptors per 1 MiB tile); y is written
    fp16 in the PE-natural layout and inverse-permuted + upcast on host.
    This halves mandatory HBM traffic vs fp32 (16+16 MiB per core) and
    removes all DMA-descriptor inefficiency (the v2 limiter: 512-B runs).
  - Per 128-token tile, indices: t = 128*o + 4c + 2h + g (c in 0..31).
      x_pre[o, p=(g,i), f=(c,h,j)] = X_t[i, j]
    Stage 1 (contract i), per c: stationary lhsT = x slice [(g,i),(h,j)],
      moving rhs = kron(I2, U) [(g,i),(g',k)]  ->  W[(h,j), (g,k)] per c
      (4 tokens per matmul; j lands on partitions so stage 2 needs no
      on-chip transpose).
    Stage 2 (contract j), per chunk q (c = 4q..4q+3): stationary
      lhsT = kron(I2, V) [(h,j),(h',l)], moving rhs = W chunk [128, 512]
      ->  Y[(h,l), (c,g,k)].
  - fp16 compute (PE 1 cycle/col), fp32 PSUM, fp16 DRAM y.
  - Engine budget per core @ target: DMA ~94 us (HBM 358 GB/s floor),
    PE ~75-88 us, DVE (W copies) ~55 us, ACT (Y copies) ~70 us.
"""

import os

import numpy as np

import concourse.bacc as bacc
import concourse.bass as bass
import concourse.mybir as mybir
import concourse.tile as tile
from concourse.bass_utils import run_bass_kernel_spmd

N_CORES = 8
B, S, D = 4, 4096, 4096
T = B * S                  # 16384 tokens
TPC = T // N_CORES         # 2048 tokens per core
TT = 128                   # tokens per tile
N_TILES = TPC // TT        # 16
O_TOT = T // TT            # 128 tiles over the full problem

F32 = mybir.dt.float32
F16 = mybir.dt.float16

LAST_RESULTS = None        # test harness can inspect exec_time_ns etc.

_CACHE: dict = {}


GT = 4                     # tiles per SBUF group (512 tokens, 4 MiB x+y)


def _build_nc(use_bias: bool, tpc: int = TPC) -> bass.Bass:
    n_grp = tpc // (GT * TT)   # 4 groups per core
    nc = bacc.Bacc()

    x = nc.dram_tensor("x", [tpc, D], F16, kind="ExternalInput")
    uu = nc.dram_tensor("uu", [128, 128], F16, kind="ExternalInput")
    vv = nc.dram_tensor("vv", [128, 128], F16, kind="ExternalInput")
    if use_bias:
        bias = nc.dram_tensor("bias", [128, 512], F32, kind="ExternalInput")
    y = nc.dram_tensor("y", [tpc, D], F16, kind="ExternalOutput")

    # 2 MiB transfers: sg subgroups x 2 tiles x 128 rows.  All input
    # transfers are issued FIRST on the sync HWDGE ring; output transfers
    # go on the same ring BEHIND them, so the ring FIFO serializes the
    # phases: reads stream solo at ~410 GB/s, then writes drain solo,
    # avoiding the HBM read/write interleave penalty and the output-
    # starvation tail.  SBUF buffers the x/y backlog (~150 KiB/partition).
    n_sg = tpc // (2 * TT)     # 8 subgroups of 2 tiles per core
    xv = x[:].rearrange("(sg o p) f -> sg p o f", o=2, p=128)
    yv = y[:].rearrange("(sg o p) f -> sg p o f", o=2, p=128)

    with tile.TileContext(nc) as tc:
        with (
            tc.tile_pool(name="consts", bufs=1) as cpool,
            tc.tile_pool(name="xt", bufs=5) as x_pool,
            tc.tile_pool(name="wt", bufs=6) as w_pool,
            tc.tile_pool(name="yo", bufs=5) as y_pool,
            tc.tile_pool(name="pw", bufs=2, space="PSUM") as pw_pool,
            tc.tile_pool(name="py", bufs=2, space="PSUM") as py_pool,
        ):
            uu_sb = cpool.tile([128, 128], F16)
            nc.sync.dma_start(out=uu_sb[:], in_=uu[:])
            vv_sb = cpool.tile([128, 128], F16)
            nc.sync.dma_start(out=vv_sb[:], in_=vv[:])
            if use_bias:
                bias_sb = cpool.tile([128, 512], F32)
                nc.sync.dma_start(out=bias_sb[:], in_=bias[:])

            xts = []
            for sg in range(n_sg):
                xt = x_pool.tile([128, 2 * 4096], F16)
                if sg == 0:
                    # split the first transfer so compute starts sooner
                    for ot in range(2):
                        nc.sync.dma_start(
                            out=xt[:, ot * 4096:(ot + 1) * 4096],
                            in_=xv[sg][:, ot],
                        )
                else:
                    nc.sync.dma_start(
                        out=xt[:].rearrange("p (o f) -> p o f", o=2),
                        in_=xv[sg],
                    )
                xts.append(xt)

            # software-pipelined emission: stage-2 of wave w-1 is emitted
            # after stage-1 of wave w so the PE FIFO never head-of-line
            # blocks on the DVE W-copy.
            n_wave = n_sg * 8          # 64 waves of 1 KiB columns
            pend = None                # (wt, yt, dst_off) awaiting stage 2
            yts = []
            for w in range(n_wave):
                sg, r = divmod(w, 8)
                if r == 0:
                    yt = y_pool.tile([128, 2 * 4096], F16)
                    yts.append(yt)
                xt = xts[sg]
                pw = pw_pool.tile([128, 1024], F32)
                for cc in range(8):
                    c = 8 * r + cc
                    nc.tensor.matmul(
                        pw[:, cc * 128:(cc + 1) * 128],
                        xt[:, c * 128:(c + 1) * 128],
                        uu_sb[:],
                        start=True,
                        stop=True,
                    )
                wt = w_pool.tile([128, 1024], F16)
                nc.vector.tensor_copy(out=wt[:], in_=pw[:])
                stages = [pend] if pend is not None else []
                pend = (wt, yt, 1024 * r)
                if w == n_wave - 1:
                    stages.append(pend)
                for wt2, yt2, off in stages:
                    py = py_pool.tile([128, 1024], F32)
                    for hh in range(2):
                        nc.tensor.matmul(
                            py[:, hh * 512:(hh + 1) * 512],
                            vv_sb[:],
                            wt2[:, hh * 512:(hh + 1) * 512],
                            start=True,
                            stop=True,
                        )
                    dst = yt2[:, off:off + 1024]
                    if use_bias:
                        nc.vector.tensor_tensor(
                            dst.rearrange("p (r f) -> p r f", r=2),
                            py[:].rearrange("p (r f) -> p r f", r=2),
                            bias_sb[:][:, None, :].to_broadcast((128, 2, 512)),
                            mybir.AluOpType.add,
                        )
                    else:
                        nc.scalar.copy(out=dst, in_=py[:])
                    if off == 7 * 1024:
                        osg = next(
                            i for i, t in enumerate(yts) if t is yt2
                        )
                        nc.sync.dma_start(
                            out=yv[osg],
                            in_=yt2[:].rearrange("p (o f) -> p o f", o=2),
                        )

    nc.finalize()
    return nc


def _make_consts(U, V, b=None) -> dict:
    U32 = np.asarray(U, dtype=np.float32)
    V32 = np.asarray(V, dtype=np.float32)
    eye2 = np.eye(2, dtype=np.float32)
    out = {
        "uu": np.kron(eye2, U32).astype(np.float16),
        "vv": np.kron(eye2, V32).astype(np.float16),
    }
    if b is not None:
        # bias_sb[(h,l), (c4,g,k)] = b[64k + l] : independent of h, c4, g.
        blk = np.asarray(b, dtype=np.float32).reshape(64, 64).T  # [l, k]
        out["bias"] = np.ascontiguousarray(
            np.tile(blk, (2, 8)).astype(np.float32)
        )
    return out


def _get_nc(use_bias: bool) -> bass.Bass:
    key = ("nc", use_bias)
    if key not in _CACHE:
        _CACHE[key] = _build_nc(use_bias)
    return _CACHE[key]


def _pre_permute(x: np.ndarray) -> np.ndarray:
    """[T, 4096] fp32 -> fp16 [O_TOT, 128, 4096] with
    x_pre[o, 64g+i, 128c+64h+j] = x[128o+4c+2h+g, 64i+j]."""
    xh = np.ascontiguousarray(x.reshape(T, D)).astype(np.float16)
    xp = xh.reshape(O_TOT, 32, 2, 2, 64, 64)       # [o, c, h, g, i, j]
    xp = xp.transpose(0, 3, 4, 1, 2, 5)            # [o, g, i, c, h, j]
    return np.ascontiguousarray(xp).reshape(O_TOT, 128, D)


def _post_permute(y_post: np.ndarray) -> np.ndarray:
    """fp16 [O_TOT, 128, 4096] with
    y_post[o, 64h+l, 128c+64g+k] = y[128o+4c+2h+g, 64k+l] -> [T, D] fp32."""
    yp = y_post.reshape(O_TOT, 2, 64, 32, 2, 64)   # [o, h, l, c, g, k]
    yp = yp.transpose(0, 3, 1, 4, 5, 2)            # [o, c, h, g, k, l]
    return np.ascontiguousarray(yp).astype(np.float32).reshape(T, D)


def kernel(x: np.ndarray, U: np.ndarray, V: np.ndarray, b: np.ndarray) -> np.ndarray:
    global LAST_RESULTS
    assert x.shape == (B, S, D) and U.shape == (64, 64) and V.shape == (64, 64)

    use_bias = bool(np.any(np.asarray(b) != 0))
    nc = _get_nc(use_bias)

    xp = _pre_permute(np.asarray(x, dtype=np.float32))
    in_map_common = _make_consts(U, V, b if use_bias else None)

    opc = N_TILES  # o-tiles per core
    in_maps = [
        {
            "x": xp[c * opc:(c + 1) * opc].reshape(TPC, D),
            **in_map_common,
        }
        for c in range(N_CORES)
    ]

    res = run_bass_kernel_spmd(
        nc,
        in_maps,
        core_ids=list(range(N_CORES)),
        trace=bool(os.environ.get("BASS_TRACE")),
    )
    LAST_RESULTS = res

    y_post = np.concatenate(
        [res.results[c]["y"].reshape(opc, 128, D) for c in range(N_CORES)],
        axis=0,
    )
    return _post_permute(y_post).reshape(B, S, D)
